# revision 33
# baseline (speedup 1.0000x reference)
"""DeepFFM Trainium2 kernel (8-core SPMD, batch-parallel).

Strategy
--------
Index-driven staging runs on the host while sharding: the 500MB FM_V tensor
is only read at 39 static indices (so the pairwise-interaction matrix S is
tiny and data-independent), and the first-order + v^T(S*mask)v terms are
0.24% of the FLOPs — both are computed on the host and shipped as one f32
row.  The MLP (99.7% of the FLOPs) runs on 8 NeuronCores, data-parallel over
the batch (2048 rows/core, four 512-column tiles).

Per-core kernel design:
- "Transposed activation" layout: activations live as [hidden, batch_cols]
  with hidden units on partitions, so every layer is a plain lhsT.T @ rhs
  matmul chain with no transposes anywhere.
- Layers 1-2 run scaled fp8e4m3 with DoubleRow pairing (K rows p/p+128 map
  to the j-dimension, which relu chunk layout provides for free), halving
  their matmul count; layer 3 and the head stay bf16.  All scales are
  powers of two, folded exactly into the relu (ACT scale= / DVE mult+max).
  L3-DoubleRow is deliberately NOT used: all-DR triggered a sustained
  half-clock power throttle.
- A scratch-matmul warmup block ramps the PE clock (HAM) to 2.4 GHz while
  the input DMAs land; inputs are packed into few wide 128-partition
  transfers issued in need-order (the HW DMA queues serve concurrent
  transfers round-robin, and few-partition transfers serialize onto a
  single queue, so layout and issue order both matter).
"""

import sys
import types

import numpy as np
import ml_dtypes


def _ensure_concourse():
    try:
        import concourse  # noqa: F401
    except ImportError:
        for p in ("/opt/trn_rl_repo", "/root/.axon_site/_ro/trn_rl_repo"):
            sys.path.insert(0, p)


def _ensure_axon_hooks_importable():
    """bass_utils imports antenv.axon_hooks unconditionally when tracing is
    requested; some images lack that module.  Provide a no-op registry so a
    trace request degrades gracefully instead of crashing."""
    try:
        import antenv
    except ImportError:
        return
    try:
        import antenv.axon_hooks  # noqa: F401
        return
    except ImportError:
        pass
    mod = types.ModuleType("antenv.axon_hooks")
    _hook = [None]
    mod.set_axon_ntff_profile_hook = lambda h: _hook.__setitem__(0, h)
    mod.get_axon_ntff_profile_hook = lambda: _hook[0]
    sys.modules["antenv.axon_hooks"] = mod
    antenv.axon_hooks = mod


_ensure_concourse()
_ensure_axon_hooks_importable()

import concourse.bass as bass  # noqa: E402
import concourse.tile as tile  # noqa: E402
from concourse import bacc, mybir  # noqa: E402
from concourse.bass import ds, ts  # noqa: E402
from concourse.bass_utils import run_bass_kernel_spmd  # noqa: E402
from concourse.tile_rust import add_dep_helper  # noqa: E402

F32 = mybir.dt.float32
BF16 = mybir.dt.bfloat16
FP8 = mybir.dt.float8e4
FP8NP = mybir.dt.np(mybir.dt.float8e4)
SCL_X = 16.0
SCL_W = 8.0
DRMODE = mybir.MatmulPerfMode.DoubleRow
AF = mybir.ActivationFunctionType
ALU = mybir.AluOpType
BF16NP = ml_dtypes.bfloat16

# Problem constants (fixed by the model definition).
FIELD_SIZE = 39
FEATURE_SIZE = 50000
N_GROUPS = 8
EMB = 8
D0 = FIELD_SIZE * EMB  # 312
N_CORES = 8
FIELD2FEATURE = np.arange(FIELD_SIZE, dtype=np.int64) * 1000
FIELD2FIELDS = np.arange(FIELD_SIZE, dtype=np.int64) % N_GROUPS

NT_COLS = 512  # batch columns per compute tile (one PSUM bank)
HALF = NT_COLS // 2
N_WARMUP = 11  # scratch matmuls at kernel start to ramp the PE clock (HAM)

FAUG = FIELD_SIZE + 2  # vals rows + linear-term row + ones row

# Column offsets of each weight block inside the packed [128, WPACK] blob.
_OFF_W0 = 0  # 3 chunks of 512
_OFF_W1 = 3 * 512  # 4 chunks of 256
_OFF_W2 = _OFF_W1 + 4 * 256  # 2 chunks of 128
_OFF_W3 = _OFF_W2 + 2 * 128  # 1 col
_OFF_MM = _OFF_W3 + 1  # FAUG cols (rows 0:FAUG)
WPACK = _OFF_MM + FAUG

_CACHE = {}


def _build_nc(bc: int, zero_bias: bool):
    """Build + compile the per-core Bass program for a batch shard of `bc`."""
    nt = bc // NT_COLS
    nc = bacc.Bacc("TRN2", target_bir_lowering=False, debug=False)

    # wpF = fp8 [w0dr | w0c | w1dr | f32 biases (28B) | tile-0 x pack]
    wpF = nc.dram_tensor(
        "wpF", [128, 2560 + 28 + 3 * NT_COLS], FP8, kind="ExternalInput"
    ).ap()
    xp = nc.dram_tensor(
        "xp", [nt - 1, 128, 3, NT_COLS], FP8, kind="ExternalInput"
    ).ap()
    lin = nc.dram_tensor("lin", [1, bc], F32, kind="ExternalInput").ap()
    wpB = nc.dram_tensor("wpB", [128, WPACK - _OFF_W1], BF16, kind="ExternalInput").ap()
    out = nc.dram_tensor("out", [1, bc], F32, kind="ExternalOutput").ap()

    with tile.TileContext(nc) as tc:
        with (
            tc.tile_pool(name="consts", bufs=1) as consts,
            tc.tile_pool(name="iox", bufs=1) as iox,
            tc.tile_pool(name="acts", bufs=3) as acts,
            tc.tile_pool(name="small", bufs=3) as small,
            tc.tile_pool(name="warm", bufs=1) as warm,
            tc.tile_pool(name="psA", bufs=6, space="PSUM") as psA,
            tc.tile_pool(name="psB", bufs=1, space="PSUM") as psB,
            tc.tile_pool(name="psC", bufs=1, space="PSUM") as psC,
        ):
            # ---- input loads, in need-order (DMA queues serve concurrent
            # transfers round-robin; issue order biases completion order) ----
            wF = consts.tile([128, 2560 + 28 + 3 * NT_COLS], FP8, tag="wF")
            nc.sync.dma_start(wF, wpF)
            wA = wF[:, 0 : 2560 + 28]
            xts = [None] + [
                iox.tile([128, 3, NT_COLS], FP8, tag=f"x{i}", name=f"xt{i}")
                for i in range(1, nt)
            ]
            wB = consts.tile([128, WPACK - _OFF_W1], BF16, tag="wB")
            nc.sync.dma_start(wB, wpB)
            lint = consts.tile([1, bc], F32, tag="lint")
            nc.sync.dma_start(lint, lin)
            for i in range(1, nt):
                nc.sync.dma_start(xts[i], xp[i - 1])
            x0 = wF[:, 2560 + 28 :].rearrange("p (j n) -> p j n", j=3)

            # ---- HAM warmup: keep the PE busy while DMAs land so the
            # clock ramps to 2.4 GHz before the first real matmul ----
            wscr = warm.tile([128, 128], BF16, tag="wscr")
            nc.vector.memset(wscr, 0.0)
            xscr = warm.tile([128, NT_COLS], BF16, tag="xscr")
            nc.vector.memset(xscr, 0.0)
            pscr = psA.tile([128, NT_COLS], F32, tag="pmlp", name="pscr")
            for _ in range(N_WARMUP):
                nc.tensor.matmul(pscr, wscr, xscr, start=True, stop=True)

            # layer-1/2 weights: DoubleRow pairs (rows p/p+128) + plain 56-row tail
            w0dr = wA[:, 0:1024].rearrange("p (j m) -> p j m", j=2)
            w0c = wA[0:56, 1024:1536]
            w1dr = wA[:, 1536:2560].rearrange("p (j m) -> p j m", j=2)

            fpt = wA[:, 2560 : 2560 + 28].bitcast(F32)
            b0t = fpt[:, 0:4]
            b1t = fpt[:, 4:6]
            b2t = fpt[:, 6:7]

            def relu_full(dst, p, bias_ap, on_vector, scale=1.0):
                if on_vector and (scale == 1.0 or zero_bias):
                    if scale == 1.0:
                        nc.vector.tensor_scalar(
                            dst, p, bias_ap, 0.0, ALU.add, ALU.max
                        )
                    else:
                        nc.vector.tensor_scalar(dst, p, scale, 0.0, ALU.mult, ALU.max)
                else:
                    nc.scalar.activation(dst, p, AF.Relu, bias=bias_ap, scale=scale)

            # ---- batch tiles, software-pipelined across tiles so each
            # stage's relu latency is covered by other tiles' matmuls:
            # emit L1(t) | L2(t-1) | L3+head(t-2) ----
            h1s, h2s, h3s = {}, {}, {}

            def xin_of(t_i):
                xt = x0 if t_i == 0 else xts[t_i]
                return xt

            def l1_stage(t_i):
                xt = xin_of(t_i)
                h1 = acts.tile([128, 4, NT_COLS], FP8, tag="h1", name=f"h1_{t_i}")
                h1s[t_i] = h1
                for m in range(4):
                    p = psA.tile([128, NT_COLS], F32, tag="pmlp", name=f"p1_{t_i}_{m}")
                    nc.tensor.matmul(
                        p,
                        w0dr[:, :, ds(m * 128, 128)],
                        xt[:, 0:2, :],
                        start=True,
                        stop=False,
                        perf_mode=DRMODE,
                    )
                    nc.tensor.matmul(
                        p,
                        w0c[:, ds(m * 128, 128)],
                        xt[0:56, 2, :],
                        start=False,
                        stop=True,
                    )
                    relu_full(
                        h1[:, m],
                        p,
                        b0t[:, ds(m, 1)],
                        on_vector=(m % 2 == 1),
                        scale=32.0 / (SCL_X * SCL_W),
                    )

            def l2_stage(t_i):
                h1 = h1s[t_i]
                h2 = acts.tile([128, 2, NT_COLS], BF16, tag="h2", name=f"h2_{t_i}")
                h2s[t_i] = h2
                for m in range(2):
                    p = psA.tile([128, NT_COLS], F32, tag="pmlp", name=f"p2_{t_i}_{m}")
                    for g in range(2):
                        nc.tensor.matmul(
                            p,
                            w1dr[:, :, ds(g * 256 + m * 128, 128)],
                            h1[:, 2 * g : 2 * g + 2, :],
                            start=(g == 0),
                            stop=(g == 1),
                            perf_mode=DRMODE,
                        )
                    relu_full(
                        h2[:, m],
                        p,
                        b1t[:, ds(m, 1)],
                        on_vector=(m == 1),
                        scale=1.0 / 256.0,
                    )

            def l3_stage(t_i):
                h2 = h2s[t_i]
                h3 = acts.tile([128, NT_COLS], BF16, tag="h3", name=f"h3_{t_i}")
                h3s[t_i] = h3
                p = psB.tile([128, NT_COLS], F32, tag="p3", name=f"p3_{t_i}")
                for k in range(2):
                    nc.tensor.matmul(
                        p, wB[:, ds(_OFF_W2 - _OFF_W1 + k * 128, 128)], h2[:, k],
                        start=(k == 0), stop=(k == 1),
                    )
                relu_full(h3, p, b2t, on_vector=False)

            def head_stage(t_i):
                cols = ts(t_i, NT_COLS)
                po = psC.tile([1, NT_COLS], F32, tag="pout", name=f"po_{t_i}")
                nc.tensor.matmul(
                    po, wB[:, ds(_OFF_W3 - _OFF_W1, 1)], h3s[t_i], start=True, stop=True
                )
                t_sum = small.tile([1, NT_COLS], F32, tag="t_sum", name=f"tsum_{t_i}")
                nc.vector.tensor_add(t_sum, po, lint[:, cols])
                o_sb = small.tile([1, NT_COLS], F32, tag="o", name=f"o_{t_i}")
                nc.scalar.activation(o_sb, t_sum, AF.Sigmoid, scale=1.0)
                nc.sync.dma_start(out[:, cols], o_sb)

            for t_i in range(nt):
                l1_stage(t_i)
                if t_i >= 1:
                    l2_stage(t_i - 1)
                if t_i >= 2:
                    l3_stage(t_i - 2)
                if t_i >= 3:
                    head_stage(t_i - 3)
            l2_stage(nt - 1)
            l3_stage(nt - 2)
            head_stage(nt - 3)
            l3_stage(nt - 1)
            head_stage(nt - 2)
            head_stage(nt - 1)

    nc.compile()
    return nc


def _prep_host(inputs):
    """Index-driven staging + layout prep on the host; returns per-core maps."""
    feat_ids = np.asarray(inputs["feat_ids"], dtype=np.int64)
    feat_vals = np.ascontiguousarray(np.asarray(inputs["feat_vals"], dtype=np.float32))
    FM_W = np.asarray(inputs["FM_W"], dtype=np.float32)
    FM_V = np.asarray(inputs["FM_V"])
    FM_B = np.asarray(inputs["FM_B"], dtype=np.float32)
    embedding = np.asarray(inputs["embedding"], dtype=np.float32)
    outW = np.asarray(inputs["outW"], dtype=np.float32)
    outB = np.asarray(inputs["outB"], dtype=np.float32)

    B = feat_ids.shape[0]
    assert B % N_CORES == 0
    bc = B // N_CORES
    assert bc % NT_COLS == 0

    # Pairwise-interaction matrix: only 39 statically indexed rows of FM_V.
    Vi = np.stack(
        [
            np.asarray(FM_V[i, FIELD2FEATURE[i]], dtype=np.float32)
            for i in range(FIELD_SIZE)
        ]
    )  # [F, G, E]
    Vg = Vi[:, FIELD2FIELDS, :]  # [F, F, E]
    S = np.einsum("ije,jie->ij", Vg, Vg).astype(np.float32)
    M = S * np.triu(np.ones((FIELD_SIZE, FIELD_SIZE), np.float32), k=1)

    # Gathers (host staging) and transposed layouts.
    XT = embedding[feat_ids].reshape(B, D0).T.astype(np.float32)  # [312, B]
    nt_total = B // NT_COLS
    xp = np.zeros((nt_total, 128, 3, NT_COLS), dtype=FP8NP)
    xv = (SCL_X * XT).reshape(312, nt_total, NT_COLS)
    xp[:, :, 0, :] = xv[0:128].transpose(1, 0, 2).astype(FP8NP)
    xp[:, :, 1, :] = xv[128:256].transpose(1, 0, 2).astype(FP8NP)
    xp[:, 0:56, 2, :] = xv[256:312].transpose(1, 0, 2).astype(FP8NP)

    # first-order + pairwise interaction terms, both dense host math
    lin = (FM_W[feat_ids] * feat_vals).sum(axis=1) + (
        FM_B.reshape(-1)[0] + outB.reshape(-1)[0]
    )  # [B]
    inter = ((feat_vals @ M) * feat_vals).sum(axis=1)  # [B]
    lin = (lin + inter).astype(np.float32)

    # fp8 pack for layer 1: w0 chunks (scaled) | f32 biases as raw bytes
    w0 = np.asarray(inputs["deepW0"], dtype=np.float32) * SCL_W
    w1f = np.asarray(inputs["deepW1"], dtype=np.float32) * SCL_W
    wq = np.zeros((128, 2560 + 28), dtype=FP8NP)
    # DoubleRow pairs: wq[p, j*512+m] = w0[j*128+p, m] for rows 0..255
    wq[:, 0:1024] = (
        w0[0:256].reshape(2, 128, 512).transpose(1, 0, 2).reshape(128, 1024)
    ).astype(FP8NP)
    wq[0:56, 1024:1536] = w0[256:312].astype(FP8NP)
    w1q = np.zeros((128, 2, 512), dtype=np.float32)
    for g in range(2):
        for j in range(2):
            w1q[:, j, g * 256 : (g + 1) * 256] = w1f[
                g * 256 + j * 128 : g * 256 + (j + 1) * 128
            ]
    wq[:, 1536:2560] = w1q.reshape(128, 1024).astype(FP8NP)

    # bf16 pack: w1 chunks | w2 chunks | w3 | M  (w0 block left zero/unused)
    wpack = np.zeros((128, WPACK), dtype=BF16NP)
    w1 = np.asarray(inputs["deepW1"], dtype=np.float32).astype(BF16NP)
    for k in range(4):
        wpack[:, _OFF_W1 + k * 256 : _OFF_W1 + (k + 1) * 256] = w1[
            k * 128 : (k + 1) * 128
        ]
    w2 = np.asarray(inputs["deepW2"], dtype=np.float32).astype(BF16NP)
    for k in range(2):
        wpack[:, _OFF_W2 + k * 128 : _OFF_W2 + (k + 1) * 128] = w2[
            k * 128 : (k + 1) * 128
        ]
    wpack[:, _OFF_W3 : _OFF_W3 + 1] = outW.astype(BF16NP)
    b0 = np.asarray(inputs["deepB0"], dtype=np.float32)
    b1 = np.asarray(inputs["deepB1"], dtype=np.float32)
    b2 = np.asarray(inputs["deepB2"], dtype=np.float32)
    zero_bias = not (np.any(b0) or np.any(b1) or np.any(b2))
    fpk = np.zeros((128, 7), dtype=np.float32)
    fpk[:, 0:4] = (32.0 * b0).reshape(4, 128).T
    fpk[:, 4:6] = b1.reshape(2, 128).T
    fpk[:, 6:7] = b2.reshape(1, 128).T
    wq[:, 2560 : 2560 + 28] = fpk.view(FP8NP)
    wpB = np.ascontiguousarray(wpack[:, _OFF_W1:])

    in_maps = []
    for c in range(N_CORES):
        cols = slice(c * bc, (c + 1) * bc)
        m = dict(wpB=wpB)
        nt_c = bc // NT_COLS
        xc = xp[c * nt_c : (c + 1) * nt_c]  # [nt_c, 128, 3, 512]
        m["wpF"] = np.ascontiguousarray(
            np.concatenate([wq, xc[0].reshape(128, 3 * NT_COLS)], axis=1)
        )
        m["xp"] = np.ascontiguousarray(xc[1:])
        m["lin"] = np.ascontiguousarray(lin[None, cols])
        in_maps.append(m)
    return in_maps, bc, zero_bias


def _run(inputs, trace=False, **kwargs):
    in_maps, bc, zero_bias = _prep_host(inputs)
    key = (bc, zero_bias)
    if key not in _CACHE:
        _CACHE[key] = _build_nc(bc, zero_bias)
    nc = _CACHE[key]
    res = run_bass_kernel_spmd(
        nc, in_maps, core_ids=list(range(N_CORES)), trace=trace, **kwargs
    )
    out = np.concatenate(
        [np.asarray(res.results[c]["out"], dtype=np.float32)[0] for c in range(N_CORES)]
    )
    return out, res


def kernel(**inputs) -> np.ndarray:
    # The first execution after a fresh compile occasionally hits a transient
    # device error under axon; retry with the cached program.
    last = None
    for _ in range(3):
        try:
            out, _ = _run(inputs)
            return out
        except Exception as e:  # noqa: BLE001
            last = e
    raise last


# revision 34
# speedup vs baseline: 1.0100x; 1.0100x over previous
"""DeepFFM Trainium2 kernel (8-core SPMD, batch-parallel).

Strategy
--------
Index-driven staging runs on the host while sharding: the 500MB FM_V tensor
is only read at 39 static indices (so the pairwise-interaction matrix S is
tiny and data-independent), and the first-order + v^T(S*mask)v terms are
0.24% of the FLOPs — both are computed on the host and shipped as one f32
row.  The MLP (99.7% of the FLOPs) runs on 8 NeuronCores, data-parallel over
the batch (2048 rows/core, four 512-column tiles).

Per-core kernel design:
- "Transposed activation" layout: activations live as [hidden, batch_cols]
  with hidden units on partitions, so every layer is a plain lhsT.T @ rhs
  matmul chain with no transposes anywhere.
- Layers 1-2 run scaled fp8e4m3 with DoubleRow pairing (K rows p/p+128 map
  to the j-dimension, which relu chunk layout provides for free), halving
  their matmul count; layer 3 and the head stay bf16.  All scales are
  powers of two, folded exactly into the relu (ACT scale= / DVE mult+max).
  L3-DoubleRow is deliberately NOT used: all-DR triggered a sustained
  half-clock power throttle.
- A scratch-matmul warmup block ramps the PE clock (HAM) to 2.4 GHz while
  the input DMAs land; inputs are packed into few wide 128-partition
  transfers issued in need-order (the HW DMA queues serve concurrent
  transfers round-robin, and few-partition transfers serialize onto a
  single queue, so layout and issue order both matter).
"""

import sys
import types

import numpy as np
import ml_dtypes


def _ensure_concourse():
    try:
        import concourse  # noqa: F401
    except ImportError:
        for p in ("/opt/trn_rl_repo", "/root/.axon_site/_ro/trn_rl_repo"):
            sys.path.insert(0, p)


def _ensure_axon_hooks_importable():
    """bass_utils imports antenv.axon_hooks unconditionally when tracing is
    requested; some images lack that module.  Provide a no-op registry so a
    trace request degrades gracefully instead of crashing."""
    try:
        import antenv
    except ImportError:
        return
    try:
        import antenv.axon_hooks  # noqa: F401
        return
    except ImportError:
        pass
    mod = types.ModuleType("antenv.axon_hooks")
    _hook = [None]
    mod.set_axon_ntff_profile_hook = lambda h: _hook.__setitem__(0, h)
    mod.get_axon_ntff_profile_hook = lambda: _hook[0]
    sys.modules["antenv.axon_hooks"] = mod
    antenv.axon_hooks = mod


_ensure_concourse()
_ensure_axon_hooks_importable()

import concourse.bass as bass  # noqa: E402
import concourse.tile as tile  # noqa: E402
from concourse import bacc, mybir  # noqa: E402
from concourse.bass import ds, ts  # noqa: E402
from concourse.bass_utils import run_bass_kernel_spmd  # noqa: E402
from concourse.tile_rust import add_dep_helper  # noqa: E402

F32 = mybir.dt.float32
BF16 = mybir.dt.bfloat16
FP8 = mybir.dt.float8e4
FP8NP = mybir.dt.np(mybir.dt.float8e4)
SCL_X = 16.0
SCL_W = 8.0
DRMODE = mybir.MatmulPerfMode.DoubleRow
AF = mybir.ActivationFunctionType
ALU = mybir.AluOpType
BF16NP = ml_dtypes.bfloat16

# Problem constants (fixed by the model definition).
FIELD_SIZE = 39
FEATURE_SIZE = 50000
N_GROUPS = 8
EMB = 8
D0 = FIELD_SIZE * EMB  # 312
N_CORES = 8
FIELD2FEATURE = np.arange(FIELD_SIZE, dtype=np.int64) * 1000
FIELD2FIELDS = np.arange(FIELD_SIZE, dtype=np.int64) % N_GROUPS

NT_COLS = 512  # batch columns per compute tile (one PSUM bank)
HALF = NT_COLS // 2
N_WARMUP = 10  # scratch matmuls at kernel start to ramp the PE clock (HAM)

FAUG = FIELD_SIZE + 2  # vals rows + linear-term row + ones row

# Column offsets of each weight block inside the packed [128, WPACK] blob.
_OFF_W0 = 0  # 3 chunks of 512
_OFF_W1 = 3 * 512  # 4 chunks of 256
_OFF_W2 = _OFF_W1 + 4 * 256  # 2 chunks of 128
_OFF_W3 = _OFF_W2 + 2 * 128  # 1 col
_OFF_MM = _OFF_W3 + 1  # FAUG cols (rows 0:FAUG)
WPACK = _OFF_MM + FAUG

_CACHE = {}


def _build_nc(bc: int, zero_bias: bool):
    """Build + compile the per-core Bass program for a batch shard of `bc`."""
    nt = bc // NT_COLS
    nc = bacc.Bacc("TRN2", target_bir_lowering=False, debug=False)

    # wpF = fp8 [w0dr | w0c | w1dr | f32 biases (28B) | tile-0 x pack]
    wpF = nc.dram_tensor(
        "wpF", [128, 2560 + 28 + 3 * NT_COLS], FP8, kind="ExternalInput"
    ).ap()
    xp = nc.dram_tensor(
        "xp", [nt - 1, 128, 3, NT_COLS], FP8, kind="ExternalInput"
    ).ap()
    lin = nc.dram_tensor("lin", [1, bc], F32, kind="ExternalInput").ap()
    wpB = nc.dram_tensor("wpB", [128, WPACK - _OFF_W1], BF16, kind="ExternalInput").ap()
    out = nc.dram_tensor("out", [1, bc], F32, kind="ExternalOutput").ap()

    with tile.TileContext(nc) as tc:
        with (
            tc.tile_pool(name="consts", bufs=1) as consts,
            tc.tile_pool(name="iox", bufs=1) as iox,
            tc.tile_pool(name="acts", bufs=3) as acts,
            tc.tile_pool(name="small", bufs=3) as small,
            tc.tile_pool(name="warm", bufs=1) as warm,
            tc.tile_pool(name="psA", bufs=6, space="PSUM") as psA,
            tc.tile_pool(name="psB", bufs=1, space="PSUM") as psB,
            tc.tile_pool(name="psC", bufs=1, space="PSUM") as psC,
        ):
            # ---- input loads, in need-order (DMA queues serve concurrent
            # transfers round-robin; issue order biases completion order) ----
            wF = consts.tile([128, 2560 + 28 + 3 * NT_COLS], FP8, tag="wF")
            nc.sync.dma_start(wF, wpF)
            wA = wF[:, 0 : 2560 + 28]
            xts = [None] + [
                iox.tile([128, 3, NT_COLS], FP8, tag=f"x{i}", name=f"xt{i}")
                for i in range(1, nt)
            ]
            wB = consts.tile([128, WPACK - _OFF_W1], BF16, tag="wB")
            nc.sync.dma_start(wB, wpB)
            lint = consts.tile([1, bc], F32, tag="lint")
            nc.sync.dma_start(lint, lin)
            for i in range(1, nt):
                nc.sync.dma_start(xts[i], xp[i - 1])
            x0 = wF[:, 2560 + 28 :].rearrange("p (j n) -> p j n", j=3)

            # ---- HAM warmup: keep the PE busy while DMAs land so the
            # clock ramps to 2.4 GHz before the first real matmul ----
            wscr = warm.tile([128, 128], BF16, tag="wscr")
            nc.vector.memset(wscr, 0.0)
            xscr = warm.tile([128, NT_COLS], BF16, tag="xscr")
            nc.vector.memset(xscr, 0.0)
            pscr = psA.tile([128, NT_COLS], F32, tag="pmlp", name="pscr")
            for _ in range(N_WARMUP):
                nc.tensor.matmul(pscr, wscr, xscr, start=True, stop=True)

            # layer-1/2 weights: DoubleRow pairs (rows p/p+128) + plain 56-row tail
            w0dr = wA[:, 0:1024].rearrange("p (j m) -> p j m", j=2)
            w0c = wA[0:56, 1024:1536]
            w1dr = wA[:, 1536:2560].rearrange("p (j m) -> p j m", j=2)

            fpt = wA[:, 2560 : 2560 + 28].bitcast(F32)
            b0t = fpt[:, 0:4]
            b1t = fpt[:, 4:6]
            b2t = fpt[:, 6:7]

            def relu_full(dst, p, bias_ap, on_vector, scale=1.0):
                if on_vector and (scale == 1.0 or zero_bias):
                    if scale == 1.0:
                        nc.vector.tensor_scalar(
                            dst, p, bias_ap, 0.0, ALU.add, ALU.max
                        )
                    else:
                        nc.vector.tensor_scalar(dst, p, scale, 0.0, ALU.mult, ALU.max)
                else:
                    nc.scalar.activation(dst, p, AF.Relu, bias=bias_ap, scale=scale)

            # ---- batch tiles, software-pipelined across tiles so each
            # stage's relu latency is covered by other tiles' matmuls:
            # emit L1(t) | L2(t-1) | L3+head(t-2) ----
            h1s, h2s, h3s = {}, {}, {}

            def xin_of(t_i):
                xt = x0 if t_i == 0 else xts[t_i]
                return xt

            def l1_stage(t_i):
                xt = xin_of(t_i)
                h1 = acts.tile([128, 4, NT_COLS], FP8, tag="h1", name=f"h1_{t_i}")
                h1s[t_i] = h1
                for m in range(4):
                    p = psA.tile([128, NT_COLS], F32, tag="pmlp", name=f"p1_{t_i}_{m}")
                    nc.tensor.matmul(
                        p,
                        w0dr[:, :, ds(m * 128, 128)],
                        xt[:, 0:2, :],
                        start=True,
                        stop=False,
                        perf_mode=DRMODE,
                    )
                    nc.tensor.matmul(
                        p,
                        w0c[:, ds(m * 128, 128)],
                        xt[0:56, 2, :],
                        start=False,
                        stop=True,
                    )
                    relu_full(
                        h1[:, m],
                        p,
                        b0t[:, ds(m, 1)],
                        on_vector=(m % 2 == 1),
                        scale=32.0 / (SCL_X * SCL_W),
                    )

            def l2_stage(t_i):
                h1 = h1s[t_i]
                h2 = acts.tile([128, 2, NT_COLS], BF16, tag="h2", name=f"h2_{t_i}")
                h2s[t_i] = h2
                for m in range(2):
                    p = psA.tile([128, NT_COLS], F32, tag="pmlp", name=f"p2_{t_i}_{m}")
                    for g in range(2):
                        nc.tensor.matmul(
                            p,
                            w1dr[:, :, ds(g * 256 + m * 128, 128)],
                            h1[:, 2 * g : 2 * g + 2, :],
                            start=(g == 0),
                            stop=(g == 1),
                            perf_mode=DRMODE,
                        )
                    relu_full(
                        h2[:, m],
                        p,
                        b1t[:, ds(m, 1)],
                        on_vector=(m == 1),
                        scale=1.0 / 256.0,
                    )

            def l3_stage(t_i):
                h2 = h2s[t_i]
                h3 = acts.tile([128, NT_COLS], BF16, tag="h3", name=f"h3_{t_i}")
                h3s[t_i] = h3
                p = psB.tile([128, NT_COLS], F32, tag="p3", name=f"p3_{t_i}")
                for k in range(2):
                    nc.tensor.matmul(
                        p, wB[:, ds(_OFF_W2 - _OFF_W1 + k * 128, 128)], h2[:, k],
                        start=(k == 0), stop=(k == 1),
                    )
                relu_full(h3, p, b2t, on_vector=False)

            def head_stage(t_i):
                cols = ts(t_i, NT_COLS)
                po = psC.tile([1, NT_COLS], F32, tag="pout", name=f"po_{t_i}")
                nc.tensor.matmul(
                    po, wB[:, ds(_OFF_W3 - _OFF_W1, 1)], h3s[t_i], start=True, stop=True
                )
                t_sum = small.tile([1, NT_COLS], F32, tag="t_sum", name=f"tsum_{t_i}")
                nc.vector.tensor_add(t_sum, po, lint[:, cols])
                o_sb = small.tile([1, NT_COLS], F32, tag="o", name=f"o_{t_i}")
                nc.scalar.activation(o_sb, t_sum, AF.Sigmoid, scale=1.0)
                nc.sync.dma_start(out[:, cols], o_sb)

            for t_i in range(nt):
                l1_stage(t_i)
                if t_i >= 1:
                    l2_stage(t_i - 1)
                if t_i >= 2:
                    l3_stage(t_i - 2)
                if t_i >= 3:
                    head_stage(t_i - 3)
            # Epilogue: interleave scratch matmuls so the PE stays busy
            # (HAM at full clock) through the pipeline drain.
            def filler(i):
                pt = psA.tile(
                    [128, NT_COLS], F32, tag="pmlp", name=f"tailscr{i}"
                )
                nc.tensor.matmul(pt, wscr, xscr, start=True, stop=True)

            l2_stage(nt - 1)
            filler(0)
            l3_stage(nt - 2)
            head_stage(nt - 3)
            filler(1)
            l3_stage(nt - 1)
            filler(2)
            head_stage(nt - 2)
            filler(3)
            head_stage(nt - 1)

    nc.compile()
    return nc


def _prep_host(inputs):
    """Index-driven staging + layout prep on the host; returns per-core maps."""
    feat_ids = np.asarray(inputs["feat_ids"], dtype=np.int64)
    feat_vals = np.ascontiguousarray(np.asarray(inputs["feat_vals"], dtype=np.float32))
    FM_W = np.asarray(inputs["FM_W"], dtype=np.float32)
    FM_V = np.asarray(inputs["FM_V"])
    FM_B = np.asarray(inputs["FM_B"], dtype=np.float32)
    embedding = np.asarray(inputs["embedding"], dtype=np.float32)
    outW = np.asarray(inputs["outW"], dtype=np.float32)
    outB = np.asarray(inputs["outB"], dtype=np.float32)

    B = feat_ids.shape[0]
    assert B % N_CORES == 0
    bc = B // N_CORES
    assert bc % NT_COLS == 0

    # Pairwise-interaction matrix: only 39 statically indexed rows of FM_V.
    Vi = np.stack(
        [
            np.asarray(FM_V[i, FIELD2FEATURE[i]], dtype=np.float32)
            for i in range(FIELD_SIZE)
        ]
    )  # [F, G, E]
    Vg = Vi[:, FIELD2FIELDS, :]  # [F, F, E]
    S = np.einsum("ije,jie->ij", Vg, Vg).astype(np.float32)
    M = S * np.triu(np.ones((FIELD_SIZE, FIELD_SIZE), np.float32), k=1)

    # Gathers (host staging) and transposed layouts.
    XT = embedding[feat_ids].reshape(B, D0).T.astype(np.float32)  # [312, B]
    nt_total = B // NT_COLS
    xp = np.zeros((nt_total, 128, 3, NT_COLS), dtype=FP8NP)
    xv = (SCL_X * XT).reshape(312, nt_total, NT_COLS)
    xp[:, :, 0, :] = xv[0:128].transpose(1, 0, 2).astype(FP8NP)
    xp[:, :, 1, :] = xv[128:256].transpose(1, 0, 2).astype(FP8NP)
    xp[:, 0:56, 2, :] = xv[256:312].transpose(1, 0, 2).astype(FP8NP)

    # first-order + pairwise interaction terms, both dense host math
    lin = (FM_W[feat_ids] * feat_vals).sum(axis=1) + (
        FM_B.reshape(-1)[0] + outB.reshape(-1)[0]
    )  # [B]
    inter = ((feat_vals @ M) * feat_vals).sum(axis=1)  # [B]
    lin = (lin + inter).astype(np.float32)

    # fp8 pack for layer 1: w0 chunks (scaled) | f32 biases as raw bytes
    w0 = np.asarray(inputs["deepW0"], dtype=np.float32) * SCL_W
    w1f = np.asarray(inputs["deepW1"], dtype=np.float32) * SCL_W
    wq = np.zeros((128, 2560 + 28), dtype=FP8NP)
    # DoubleRow pairs: wq[p, j*512+m] = w0[j*128+p, m] for rows 0..255
    wq[:, 0:1024] = (
        w0[0:256].reshape(2, 128, 512).transpose(1, 0, 2).reshape(128, 1024)
    ).astype(FP8NP)
    wq[0:56, 1024:1536] = w0[256:312].astype(FP8NP)
    w1q = np.zeros((128, 2, 512), dtype=np.float32)
    for g in range(2):
        for j in range(2):
            w1q[:, j, g * 256 : (g + 1) * 256] = w1f[
                g * 256 + j * 128 : g * 256 + (j + 1) * 128
            ]
    wq[:, 1536:2560] = w1q.reshape(128, 1024).astype(FP8NP)

    # bf16 pack: w1 chunks | w2 chunks | w3 | M  (w0 block left zero/unused)
    wpack = np.zeros((128, WPACK), dtype=BF16NP)
    w1 = np.asarray(inputs["deepW1"], dtype=np.float32).astype(BF16NP)
    for k in range(4):
        wpack[:, _OFF_W1 + k * 256 : _OFF_W1 + (k + 1) * 256] = w1[
            k * 128 : (k + 1) * 128
        ]
    w2 = np.asarray(inputs["deepW2"], dtype=np.float32).astype(BF16NP)
    for k in range(2):
        wpack[:, _OFF_W2 + k * 128 : _OFF_W2 + (k + 1) * 128] = w2[
            k * 128 : (k + 1) * 128
        ]
    wpack[:, _OFF_W3 : _OFF_W3 + 1] = outW.astype(BF16NP)
    b0 = np.asarray(inputs["deepB0"], dtype=np.float32)
    b1 = np.asarray(inputs["deepB1"], dtype=np.float32)
    b2 = np.asarray(inputs["deepB2"], dtype=np.float32)
    zero_bias = not (np.any(b0) or np.any(b1) or np.any(b2))
    fpk = np.zeros((128, 7), dtype=np.float32)
    fpk[:, 0:4] = (32.0 * b0).reshape(4, 128).T
    fpk[:, 4:6] = b1.reshape(2, 128).T
    fpk[:, 6:7] = b2.reshape(1, 128).T
    wq[:, 2560 : 2560 + 28] = fpk.view(FP8NP)
    wpB = np.ascontiguousarray(wpack[:, _OFF_W1:])

    in_maps = []
    for c in range(N_CORES):
        cols = slice(c * bc, (c + 1) * bc)
        m = dict(wpB=wpB)
        nt_c = bc // NT_COLS
        xc = xp[c * nt_c : (c + 1) * nt_c]  # [nt_c, 128, 3, 512]
        m["wpF"] = np.ascontiguousarray(
            np.concatenate([wq, xc[0].reshape(128, 3 * NT_COLS)], axis=1)
        )
        m["xp"] = np.ascontiguousarray(xc[1:])
        m["lin"] = np.ascontiguousarray(lin[None, cols])
        in_maps.append(m)
    return in_maps, bc, zero_bias


def _run(inputs, trace=False, **kwargs):
    in_maps, bc, zero_bias = _prep_host(inputs)
    key = (bc, zero_bias)
    if key not in _CACHE:
        _CACHE[key] = _build_nc(bc, zero_bias)
    nc = _CACHE[key]
    res = run_bass_kernel_spmd(
        nc, in_maps, core_ids=list(range(N_CORES)), trace=trace, **kwargs
    )
    out = np.concatenate(
        [np.asarray(res.results[c]["out"], dtype=np.float32)[0] for c in range(N_CORES)]
    )
    return out, res


def kernel(**inputs) -> np.ndarray:
    # The first execution after a fresh compile occasionally hits a transient
    # device error under axon; retry with the cached program.
    last = None
    for _ in range(3):
        try:
            out, _ = _run(inputs)
            return out
        except Exception as e:  # noqa: BLE001
            last = e
    raise last


# revision 35
# speedup vs baseline: 1.0125x; 1.0025x over previous
"""DeepFFM Trainium2 kernel (8-core SPMD, batch-parallel).

Strategy
--------
Index-driven staging runs on the host while sharding: the 500MB FM_V tensor
is only read at 39 static indices (so the pairwise-interaction matrix S is
tiny and data-independent), and the first-order + v^T(S*mask)v terms are
0.24% of the FLOPs — both are computed on the host and shipped as one f32
row.  The MLP (99.7% of the FLOPs) runs on 8 NeuronCores, data-parallel over
the batch (2048 rows/core, four 512-column tiles).

Per-core kernel design:
- "Transposed activation" layout: activations live as [hidden, batch_cols]
  with hidden units on partitions, so every layer is a plain lhsT.T @ rhs
  matmul chain with no transposes anywhere.
- Layers 1-2 run scaled fp8e4m3 with DoubleRow pairing (K rows p/p+128 map
  to the j-dimension, which relu chunk layout provides for free), halving
  their matmul count; layer 3 and the head stay bf16.  All scales are
  powers of two, folded exactly into the relu (ACT scale= / DVE mult+max).
  L3-DoubleRow is deliberately NOT used: all-DR triggered a sustained
  half-clock power throttle.
- A scratch-matmul warmup block ramps the PE clock (HAM) to 2.4 GHz while
  the input DMAs land; inputs are packed into few wide 128-partition
  transfers issued in need-order (the HW DMA queues serve concurrent
  transfers round-robin, and few-partition transfers serialize onto a
  single queue, so layout and issue order both matter).
"""

import sys
import types

import numpy as np
import ml_dtypes


def _ensure_concourse():
    try:
        import concourse  # noqa: F401
    except ImportError:
        for p in ("/opt/trn_rl_repo", "/root/.axon_site/_ro/trn_rl_repo"):
            sys.path.insert(0, p)


def _ensure_axon_hooks_importable():
    """bass_utils imports antenv.axon_hooks unconditionally when tracing is
    requested; some images lack that module.  Provide a no-op registry so a
    trace request degrades gracefully instead of crashing."""
    try:
        import antenv
    except ImportError:
        return
    try:
        import antenv.axon_hooks  # noqa: F401
        return
    except ImportError:
        pass
    mod = types.ModuleType("antenv.axon_hooks")
    _hook = [None]
    mod.set_axon_ntff_profile_hook = lambda h: _hook.__setitem__(0, h)
    mod.get_axon_ntff_profile_hook = lambda: _hook[0]
    sys.modules["antenv.axon_hooks"] = mod
    antenv.axon_hooks = mod


_ensure_concourse()
_ensure_axon_hooks_importable()

import concourse.bass as bass  # noqa: E402
import concourse.tile as tile  # noqa: E402
from concourse import bacc, mybir  # noqa: E402
from concourse.bass import ds, ts  # noqa: E402
from concourse.bass_utils import run_bass_kernel_spmd  # noqa: E402
from concourse.tile_rust import add_dep_helper  # noqa: E402

F32 = mybir.dt.float32
BF16 = mybir.dt.bfloat16
FP8 = mybir.dt.float8e4
FP8NP = mybir.dt.np(mybir.dt.float8e4)
SCL_X = 16.0
SCL_W = 8.0
DRMODE = mybir.MatmulPerfMode.DoubleRow
AF = mybir.ActivationFunctionType
ALU = mybir.AluOpType
BF16NP = ml_dtypes.bfloat16

# Problem constants (fixed by the model definition).
FIELD_SIZE = 39
FEATURE_SIZE = 50000
N_GROUPS = 8
EMB = 8
D0 = FIELD_SIZE * EMB  # 312
N_CORES = 8
FIELD2FEATURE = np.arange(FIELD_SIZE, dtype=np.int64) * 1000
FIELD2FIELDS = np.arange(FIELD_SIZE, dtype=np.int64) % N_GROUPS

NT_COLS = 512  # batch columns per compute tile (one PSUM bank)
HALF = NT_COLS // 2
N_WARMUP = 11  # scratch matmuls at kernel start to ramp the PE clock (HAM)

FAUG = FIELD_SIZE + 2  # vals rows + linear-term row + ones row

# Column offsets of each weight block inside the packed [128, WPACK] blob.
_OFF_W0 = 0  # 3 chunks of 512
_OFF_W1 = 3 * 512  # 4 chunks of 256
_OFF_W2 = _OFF_W1 + 4 * 256  # 2 chunks of 128
_OFF_W3 = _OFF_W2 + 2 * 128  # 1 col
_OFF_MM = _OFF_W3 + 1  # FAUG cols (rows 0:FAUG)
WPACK = _OFF_MM + FAUG

_CACHE = {}


def _build_nc(bc: int, zero_bias: bool):
    """Build + compile the per-core Bass program for a batch shard of `bc`."""
    nt = bc // NT_COLS
    nc = bacc.Bacc("TRN2", target_bir_lowering=False, debug=False)

    # wpF = fp8 [w0dr | w0c | w1dr | f32 biases (28B) | tile-0 x pack]
    wpF = nc.dram_tensor(
        "wpF", [128, 2560 + 28 + 3 * NT_COLS], FP8, kind="ExternalInput"
    ).ap()
    xp = nc.dram_tensor(
        "xp", [nt - 1, 128, 3, NT_COLS], FP8, kind="ExternalInput"
    ).ap()
    lin = nc.dram_tensor("lin", [1, bc], F32, kind="ExternalInput").ap()
    wpB = nc.dram_tensor("wpB", [128, WPACK - _OFF_W1], BF16, kind="ExternalInput").ap()
    out = nc.dram_tensor("out", [1, bc], F32, kind="ExternalOutput").ap()

    with tile.TileContext(nc) as tc:
        with (
            tc.tile_pool(name="consts", bufs=1) as consts,
            tc.tile_pool(name="iox", bufs=1) as iox,
            tc.tile_pool(name="acts", bufs=3) as acts,
            tc.tile_pool(name="small", bufs=3) as small,
            tc.tile_pool(name="warm", bufs=1) as warm,
            tc.tile_pool(name="psA", bufs=6, space="PSUM") as psA,
            tc.tile_pool(name="psB", bufs=1, space="PSUM") as psB,
            tc.tile_pool(name="psC", bufs=1, space="PSUM") as psC,
        ):
            # ---- input loads, in need-order (DMA queues serve concurrent
            # transfers round-robin; issue order biases completion order) ----
            wF = consts.tile([128, 2560 + 28 + 3 * NT_COLS], FP8, tag="wF")
            nc.sync.dma_start(wF, wpF)
            wA = wF[:, 0 : 2560 + 28]
            xts = [None] + [
                iox.tile([128, 3, NT_COLS], FP8, tag=f"x{i}", name=f"xt{i}")
                for i in range(1, nt)
            ]
            wB = consts.tile([128, WPACK - _OFF_W1], BF16, tag="wB")
            nc.sync.dma_start(wB, wpB)
            lint = consts.tile([1, bc], F32, tag="lint")
            nc.sync.dma_start(lint, lin)
            for i in range(1, nt):
                nc.sync.dma_start(xts[i], xp[i - 1])
            x0 = wF[:, 2560 + 28 :].rearrange("p (j n) -> p j n", j=3)

            # ---- HAM warmup: keep the PE busy while DMAs land so the
            # clock ramps to 2.4 GHz before the first real matmul ----
            wscr = warm.tile([128, 128], BF16, tag="wscr")
            nc.vector.memset(wscr, 0.0)
            xscr = warm.tile([128, NT_COLS], BF16, tag="xscr")
            nc.vector.memset(xscr, 0.0)
            pscr = psA.tile([128, NT_COLS], F32, tag="pmlp", name="pscr")
            for _ in range(N_WARMUP):
                nc.tensor.matmul(pscr, wscr, xscr, start=True, stop=True)

            # layer-1/2 weights: DoubleRow pairs (rows p/p+128) + plain 56-row tail
            w0dr = wA[:, 0:1024].rearrange("p (j m) -> p j m", j=2)
            w0c = wA[0:56, 1024:1536]
            w1dr = wA[:, 1536:2560].rearrange("p (j m) -> p j m", j=2)

            fpt = wA[:, 2560 : 2560 + 28].bitcast(F32)
            b0t = fpt[:, 0:4]
            b1t = fpt[:, 4:6]
            b2t = fpt[:, 6:7]

            def relu_full(dst, p, bias_ap, on_vector, scale=1.0):
                if on_vector and (scale == 1.0 or zero_bias):
                    if scale == 1.0:
                        nc.vector.tensor_scalar(
                            dst, p, bias_ap, 0.0, ALU.add, ALU.max
                        )
                    else:
                        nc.vector.tensor_scalar(dst, p, scale, 0.0, ALU.mult, ALU.max)
                else:
                    nc.scalar.activation(dst, p, AF.Relu, bias=bias_ap, scale=scale)

            # ---- batch tiles, software-pipelined across tiles so each
            # stage's relu latency is covered by other tiles' matmuls:
            # emit L1(t) | L2(t-1) | L3+head(t-2) ----
            h1s, h2s, h3s = {}, {}, {}

            def xin_of(t_i):
                xt = x0 if t_i == 0 else xts[t_i]
                return xt

            def l1_stage(t_i):
                xt = xin_of(t_i)
                h1 = acts.tile([128, 4, NT_COLS], FP8, tag="h1", name=f"h1_{t_i}")
                h1s[t_i] = h1
                for m in range(4):
                    p = psA.tile([128, NT_COLS], F32, tag="pmlp", name=f"p1_{t_i}_{m}")
                    nc.tensor.matmul(
                        p,
                        w0dr[:, :, ds(m * 128, 128)],
                        xt[:, 0:2, :],
                        start=True,
                        stop=False,
                        perf_mode=DRMODE,
                    )
                    nc.tensor.matmul(
                        p,
                        w0c[:, ds(m * 128, 128)],
                        xt[0:56, 2, :],
                        start=False,
                        stop=True,
                    )
                    relu_full(
                        h1[:, m],
                        p,
                        b0t[:, ds(m, 1)],
                        on_vector=(m % 2 == 1),
                        scale=32.0 / (SCL_X * SCL_W),
                    )

            def l2_stage(t_i):
                h1 = h1s[t_i]
                h2 = acts.tile([128, 2, NT_COLS], BF16, tag="h2", name=f"h2_{t_i}")
                h2s[t_i] = h2
                for m in range(2):
                    p = psA.tile([128, NT_COLS], F32, tag="pmlp", name=f"p2_{t_i}_{m}")
                    for g in range(2):
                        nc.tensor.matmul(
                            p,
                            w1dr[:, :, ds(g * 256 + m * 128, 128)],
                            h1[:, 2 * g : 2 * g + 2, :],
                            start=(g == 0),
                            stop=(g == 1),
                            perf_mode=DRMODE,
                        )
                    relu_full(
                        h2[:, m],
                        p,
                        b1t[:, ds(m, 1)],
                        on_vector=(m == 1),
                        scale=1.0 / 256.0,
                    )

            def l3_stage(t_i):
                h2 = h2s[t_i]
                h3 = acts.tile([128, NT_COLS], BF16, tag="h3", name=f"h3_{t_i}")
                h3s[t_i] = h3
                p = psB.tile([128, NT_COLS], F32, tag="p3", name=f"p3_{t_i}")
                for k in range(2):
                    nc.tensor.matmul(
                        p, wB[:, ds(_OFF_W2 - _OFF_W1 + k * 128, 128)], h2[:, k],
                        start=(k == 0), stop=(k == 1),
                    )
                relu_full(h3, p, b2t, on_vector=False)

            def head_stage(t_i):
                cols = ts(t_i, NT_COLS)
                po = psC.tile([1, NT_COLS], F32, tag="pout", name=f"po_{t_i}")
                nc.tensor.matmul(
                    po, wB[:, ds(_OFF_W3 - _OFF_W1, 1)], h3s[t_i], start=True, stop=True
                )
                t_sum = small.tile([1, NT_COLS], F32, tag="t_sum", name=f"tsum_{t_i}")
                nc.vector.tensor_add(t_sum, po, lint[:, cols])
                o_sb = small.tile([1, NT_COLS], F32, tag="o", name=f"o_{t_i}")
                nc.scalar.activation(o_sb, t_sum, AF.Sigmoid, scale=1.0)
                nc.sync.dma_start(out[:, cols], o_sb)

            for t_i in range(nt):
                l1_stage(t_i)
                if t_i >= 1:
                    l2_stage(t_i - 1)
                if t_i >= 2:
                    l3_stage(t_i - 2)
                if t_i >= 3:
                    head_stage(t_i - 3)
            # Epilogue: interleave scratch matmuls so the PE stays busy
            # (HAM at full clock) through the pipeline drain.
            def filler(i):
                pt = psA.tile(
                    [128, NT_COLS], F32, tag="pmlp", name=f"tailscr{i}"
                )
                nc.tensor.matmul(pt, wscr, xscr, start=True, stop=True)

            l2_stage(nt - 1)
            filler(0)
            l3_stage(nt - 2)
            head_stage(nt - 3)
            filler(1)
            l3_stage(nt - 1)
            filler(2)
            head_stage(nt - 2)
            filler(3)
            head_stage(nt - 1)

    nc.compile()
    return nc


def _prep_host(inputs):
    """Index-driven staging + layout prep on the host; returns per-core maps."""
    feat_ids = np.asarray(inputs["feat_ids"], dtype=np.int64)
    feat_vals = np.ascontiguousarray(np.asarray(inputs["feat_vals"], dtype=np.float32))
    FM_W = np.asarray(inputs["FM_W"], dtype=np.float32)
    FM_V = np.asarray(inputs["FM_V"])
    FM_B = np.asarray(inputs["FM_B"], dtype=np.float32)
    embedding = np.asarray(inputs["embedding"], dtype=np.float32)
    outW = np.asarray(inputs["outW"], dtype=np.float32)
    outB = np.asarray(inputs["outB"], dtype=np.float32)

    B = feat_ids.shape[0]
    assert B % N_CORES == 0
    bc = B // N_CORES
    assert bc % NT_COLS == 0

    # Pairwise-interaction matrix: only 39 statically indexed rows of FM_V.
    Vi = np.stack(
        [
            np.asarray(FM_V[i, FIELD2FEATURE[i]], dtype=np.float32)
            for i in range(FIELD_SIZE)
        ]
    )  # [F, G, E]
    Vg = Vi[:, FIELD2FIELDS, :]  # [F, F, E]
    S = np.einsum("ije,jie->ij", Vg, Vg).astype(np.float32)
    M = S * np.triu(np.ones((FIELD_SIZE, FIELD_SIZE), np.float32), k=1)

    # Gathers (host staging) and transposed layouts.
    XT = embedding[feat_ids].reshape(B, D0).T.astype(np.float32)  # [312, B]
    nt_total = B // NT_COLS
    xp = np.zeros((nt_total, 128, 3, NT_COLS), dtype=FP8NP)
    xv = (SCL_X * XT).reshape(312, nt_total, NT_COLS)
    xp[:, :, 0, :] = xv[0:128].transpose(1, 0, 2).astype(FP8NP)
    xp[:, :, 1, :] = xv[128:256].transpose(1, 0, 2).astype(FP8NP)
    xp[:, 0:56, 2, :] = xv[256:312].transpose(1, 0, 2).astype(FP8NP)

    # first-order + pairwise interaction terms, both dense host math
    lin = (FM_W[feat_ids] * feat_vals).sum(axis=1) + (
        FM_B.reshape(-1)[0] + outB.reshape(-1)[0]
    )  # [B]
    inter = ((feat_vals @ M) * feat_vals).sum(axis=1)  # [B]
    lin = (lin + inter).astype(np.float32)

    # fp8 pack for layer 1: w0 chunks (scaled) | f32 biases as raw bytes
    w0 = np.asarray(inputs["deepW0"], dtype=np.float32) * SCL_W
    w1f = np.asarray(inputs["deepW1"], dtype=np.float32) * SCL_W
    wq = np.zeros((128, 2560 + 28), dtype=FP8NP)
    # DoubleRow pairs: wq[p, j*512+m] = w0[j*128+p, m] for rows 0..255
    wq[:, 0:1024] = (
        w0[0:256].reshape(2, 128, 512).transpose(1, 0, 2).reshape(128, 1024)
    ).astype(FP8NP)
    wq[0:56, 1024:1536] = w0[256:312].astype(FP8NP)
    w1q = np.zeros((128, 2, 512), dtype=np.float32)
    for g in range(2):
        for j in range(2):
            w1q[:, j, g * 256 : (g + 1) * 256] = w1f[
                g * 256 + j * 128 : g * 256 + (j + 1) * 128
            ]
    wq[:, 1536:2560] = w1q.reshape(128, 1024).astype(FP8NP)

    # bf16 pack: w1 chunks | w2 chunks | w3 | M  (w0 block left zero/unused)
    wpack = np.zeros((128, WPACK), dtype=BF16NP)
    w1 = np.asarray(inputs["deepW1"], dtype=np.float32).astype(BF16NP)
    for k in range(4):
        wpack[:, _OFF_W1 + k * 256 : _OFF_W1 + (k + 1) * 256] = w1[
            k * 128 : (k + 1) * 128
        ]
    w2 = np.asarray(inputs["deepW2"], dtype=np.float32).astype(BF16NP)
    for k in range(2):
        wpack[:, _OFF_W2 + k * 128 : _OFF_W2 + (k + 1) * 128] = w2[
            k * 128 : (k + 1) * 128
        ]
    wpack[:, _OFF_W3 : _OFF_W3 + 1] = outW.astype(BF16NP)
    b0 = np.asarray(inputs["deepB0"], dtype=np.float32)
    b1 = np.asarray(inputs["deepB1"], dtype=np.float32)
    b2 = np.asarray(inputs["deepB2"], dtype=np.float32)
    zero_bias = not (np.any(b0) or np.any(b1) or np.any(b2))
    fpk = np.zeros((128, 7), dtype=np.float32)
    fpk[:, 0:4] = (32.0 * b0).reshape(4, 128).T
    fpk[:, 4:6] = b1.reshape(2, 128).T
    fpk[:, 6:7] = b2.reshape(1, 128).T
    wq[:, 2560 : 2560 + 28] = fpk.view(FP8NP)
    wpB = np.ascontiguousarray(wpack[:, _OFF_W1:])

    in_maps = []
    for c in range(N_CORES):
        cols = slice(c * bc, (c + 1) * bc)
        m = dict(wpB=wpB)
        nt_c = bc // NT_COLS
        xc = xp[c * nt_c : (c + 1) * nt_c]  # [nt_c, 128, 3, 512]
        m["wpF"] = np.ascontiguousarray(
            np.concatenate([wq, xc[0].reshape(128, 3 * NT_COLS)], axis=1)
        )
        m["xp"] = np.ascontiguousarray(xc[1:])
        m["lin"] = np.ascontiguousarray(lin[None, cols])
        in_maps.append(m)
    return in_maps, bc, zero_bias


def _run(inputs, trace=False, **kwargs):
    in_maps, bc, zero_bias = _prep_host(inputs)
    key = (bc, zero_bias)
    if key not in _CACHE:
        _CACHE[key] = _build_nc(bc, zero_bias)
    nc = _CACHE[key]
    res = run_bass_kernel_spmd(
        nc, in_maps, core_ids=list(range(N_CORES)), trace=trace, **kwargs
    )
    out = np.concatenate(
        [np.asarray(res.results[c]["out"], dtype=np.float32)[0] for c in range(N_CORES)]
    )
    return out, res


def kernel(**inputs) -> np.ndarray:
    # The first execution after a fresh compile occasionally hits a transient
    # device error under axon; retry with the cached program.
    last = None
    for _ in range(3):
        try:
            out, _ = _run(inputs)
            return out
        except Exception as e:  # noqa: BLE001
            last = e
    raise last
